# revision 3
# baseline (speedup 1.0000x reference)
"""2-layer GraphSAGE (mean agg) + linear head on 8 TRN2 NeuronCores.

Sharding: dst nodes split evenly across 8 cores (6250 each). Each core
processes the edges terminating in its node range. Per layer:
  - gather h[src] from the full (replicated) table via SWDGE indirect DMA,
    128 edges per instruction (one row index per partition)
  - segment-mean via one-hot matmul: S[e, j] = (dst_loc[e]==j) * 1/deg,
    accumulated in PSUM as meanT[feat, 512-dst-segment] += G.T @ S
  - h_outT = relu(Wl.T @ meanT + Wr.T @ prevT + b) per 512-node strip
  - layer 1 output is transposed to node-major and AllGathered into the
    full table for layer 2's gathers.
Head y = h2 @ Wlin + blin computed per 128-node block on PE.
"""
import sys
sys.path.insert(0, "/opt/trn_rl_repo")
import numpy as np

N = 50000
E = 800000
D = 128
NCORES = 8
NPC = N // NCORES            # 6250 nodes per core
SEG = 512
NSEG = (NPC + SEG - 1) // SEG   # 13 (12*512 + 106)
CH = 128                      # edges per gather chunk
PAD_OFF = 600.0               # dst-offset sentinel for pad slots

_compiled = {}
LAST = None   # (nc, in_maps) stash for external timing harnesses


def _prep(edge_index, inv_deg):
    """Chunk each core's edges: [128, NCH] src/dstoff/invdeg meta arrays."""
    src = edge_index[0].astype(np.int64)
    dst = edge_index[1].astype(np.int64)
    order = np.argsort(dst, kind="stable")
    s_src = src[order]
    s_dst = dst[order]
    core_lo = np.searchsorted(s_dst, np.arange(NCORES) * NPC)
    core_hi = np.searchsorted(s_dst, (np.arange(NCORES) + 1) * NPC)

    per = []
    cnt = np.zeros((NCORES, NSEG), dtype=np.int64)
    for c in range(NCORES):
        cs = s_src[core_lo[c]:core_hi[c]]
        cd = s_dst[core_lo[c]:core_hi[c]] - c * NPC
        segb = np.searchsorted(cd, np.arange(NSEG + 1) * SEG)
        segs = []
        for s in range(NSEG):
            a, b = segb[s], segb[s + 1]
            es, ed = cs[a:b], cd[a:b] - s * SEG
            so = np.argsort(es, kind="stable")   # src order: DRAM locality
            segs.append((es[so], ed[so]))
            cnt[c, s] = b - a
        per.append(segs)

    nch = [max(1, int(np.ceil(cnt[:, s].max() / CH))) for s in range(NSEG)]
    NCH = int(sum(nch))

    metas = []
    for c in range(NCORES):
        mi = np.zeros((CH, NCH), np.int32)
        mo = np.full((CH, NCH), PAD_OFF, np.float32)
        mv = np.zeros((CH, NCH), np.float32)
        col = 0
        for s in range(NSEG):
            es, ed = per[c][s]
            n = len(es)
            K = nch[s] * CH
            psrc = np.zeros(K, np.int64)
            psrc[:n] = es
            poff = np.full(K, PAD_OFF, np.float32)
            poff[:n] = ed
            pinv = np.zeros(K, np.float32)
            pinv[:n] = inv_deg[c * NPC + s * SEG + ed]
            mi[:, col:col + nch[s]] = psrc.reshape(nch[s], CH).T
            mo[:, col:col + nch[s]] = poff.reshape(nch[s], CH).T
            mv[:, col:col + nch[s]] = pinv.reshape(nch[s], CH).T
            col += nch[s]
        metas.append((mi, mo, mv))
    return nch, metas


def _build(nch):
    from concourse import bass, bacc, tile
    from concourse import mybir
    from concourse.masks import make_identity

    NCH = int(sum(nch))
    WID = NSEG * SEG            # 6656 padded node columns per core

    nc = bacc.Bacc("TRN2", target_bir_lowering=False, debug=False,
                   num_devices=NCORES)
    f32 = mybir.dt.float32
    xtab = nc.dram_tensor("xtab", [N, D], f32, kind="ExternalInput")
    xown = nc.dram_tensor("xown", [NPC, D], f32, kind="ExternalInput")
    msrc = nc.dram_tensor("msrc", [CH, NCH], mybir.dt.int32,
                          kind="ExternalInput")
    moff = nc.dram_tensor("moff", [CH, NCH], f32, kind="ExternalInput")
    minv = nc.dram_tensor("minv", [CH, NCH], f32, kind="ExternalInput")
    jt = nc.dram_tensor("jt", [128, SEG], f32, kind="ExternalInput")
    w1l = nc.dram_tensor("w1l", [D, D], f32, kind="ExternalInput")
    w1r = nc.dram_tensor("w1r", [D, D], f32, kind="ExternalInput")
    b1 = nc.dram_tensor("b1", [D, 1], f32, kind="ExternalInput")
    w2l = nc.dram_tensor("w2l", [D, D], f32, kind="ExternalInput")
    w2r = nc.dram_tensor("w2r", [D, D], f32, kind="ExternalInput")
    b2 = nc.dram_tensor("b2", [D, 1], f32, kind="ExternalInput")
    wlin = nc.dram_tensor("wlin", [D, 1], f32, kind="ExternalInput")
    blin = nc.dram_tensor("blin", [128, 1], f32, kind="ExternalInput")

    ag_in = nc.dram_tensor("ag_in", [NPC, D], f32)
    ag_sh = nc.dram_tensor("ag_sh", [N, D], f32, addr_space="Shared")
    h_out = nc.dram_tensor("h_out", [128, WID], f32, kind="ExternalOutput")
    y_out = nc.dram_tensor("y_out", [128, NSEG * 4], f32,
                           kind="ExternalOutput")
    NB = (NPC + 127) // 128     # 49 node blocks of 128

    with tile.TileContext(nc) as tc:
        with tc.tile_pool(name="meta", bufs=1) as mp, \
             tc.tile_pool(name="big", bufs=1) as bp, \
             tc.tile_pool(name="xin", bufs=4) as xp, \
             tc.tile_pool(name="g", bufs=6) as gp, \
             tc.tile_pool(name="s", bufs=6) as sp, \
             tc.tile_pool(name="ev", bufs=4) as ep, \
             tc.tile_pool(name="agg", bufs=2, space="PSUM") as aggp, \
             tc.tile_pool(name="tp", bufs=2, space="PSUM") as tpp, \
             tc.tile_pool(name="wm", bufs=2, space="PSUM") as wmp:

            def load(name, dram, shape, dt=f32):
                t = mp.tile(shape, dt, name=name, tag=name)
                nc.sync.dma_start(out=t[:], in_=dram[:])
                return t

            msrc_sb = load("msrc_sb", msrc, [CH, NCH], mybir.dt.int32)
            moff_sb = load("moff_sb", moff, [CH, NCH])
            minv_sb = load("minv_sb", minv, [CH, NCH])
            j_sb = load("j_sb", jt, [128, SEG])
            w1l_sb = load("w1l_sb", w1l, [D, D])
            w1r_sb = load("w1r_sb", w1r, [D, D])
            b1_sb = load("b1_sb", b1, [D, 1])
            w2l_sb = load("w2l_sb", w2l, [D, D])
            w2r_sb = load("w2r_sb", w2r, [D, D])
            b2_sb = load("b2_sb", b2, [D, 1])
            wlin_sb = load("wlin_sb", wlin, [D, 1])
            blin_sb = load("blin_sb", blin, [128, 1])

            ident = mp.tile([128, 128], f32, name="ident", tag="ident")
            make_identity(nc, ident[:])

            xT = bp.tile([128, WID], f32, name="xT", tag="xT")
            h1T = bp.tile([128, WID], f32, name="h1T", tag="h1T")
            h2T = bp.tile([128, WID], f32, name="h2T", tag="h2T")
            aggT = bp.tile([128, WID], f32, name="aggT", tag="aggT")
            yrow = bp.tile([128, NSEG * 4], f32, name="yrow", tag="yrow")

            # ---- xT: transpose own x slice ----
            for nb in range(NB):
                rows = min(128, NPC - 128 * nb)
                xin = xp.tile([128, D], f32, name=f"xin{nb}", tag="xin")
                nc.sync.dma_start(out=xin[:rows, :],
                                  in_=xown[128 * nb:128 * nb + rows, :])
                tp = tpp.tile([128, 128], f32, name=f"tp{nb}", tag="tp")
                nc.tensor.transpose(out=tp[:, :rows], in_=xin[:rows, :],
                                    identity=ident[:rows, :rows])
                nc.scalar.activation(out=xT[:, 128 * nb:128 * nb + rows],
                                     in_=tp[:, :rows],
                                     func=mybir.ActivationFunctionType.Copy)

            def layer(table, prevT, wl_sb, wr_sb, bias_sb, outT, tagp):
                col = 0
                for s in range(NSEG):
                    acc = aggp.tile([128, SEG], f32, name=f"acc{tagp}{s}",
                                    tag="acc")
                    for k in range(nch[s]):
                        g = col + k
                        gt = gp.tile([128, D], f32, name=f"g{tagp}{g}",
                                     tag="g")
                        nc.gpsimd.indirect_dma_start(
                            out=gt[:], out_offset=None, in_=table[:],
                            in_offset=bass.IndirectOffsetOnAxis(
                                ap=msrc_sb[:, g:g + 1], axis=0))
                        st = sp.tile([128, SEG], f32, name=f"s{tagp}{g}",
                                     tag="s")
                        nc.vector.tensor_scalar(
                            out=st[:], in0=j_sb[:],
                            scalar1=moff_sb[:, g:g + 1],
                            scalar2=minv_sb[:, g:g + 1],
                            op0=mybir.AluOpType.is_equal,
                            op1=mybir.AluOpType.mult)
                        nc.tensor.matmul(out=acc[:], lhsT=gt[:], rhs=st[:],
                                         start=(k == 0),
                                         stop=(k == nch[s] - 1))
                    nc.scalar.activation(
                        out=aggT[:, s * SEG:(s + 1) * SEG], in_=acc[:],
                        func=mybir.ActivationFunctionType.Copy)
                    col += nch[s]
                for s in range(NSEG):
                    w = min(SEG, NPC - s * SEG)
                    wm = wmp.tile([128, SEG], f32, name=f"wm{tagp}{s}",
                                  tag="wm")
                    nc.tensor.matmul(out=wm[:, :w], lhsT=wl_sb[:],
                                     rhs=aggT[:, s * SEG:s * SEG + w],
                                     start=True, stop=False)
                    nc.tensor.matmul(out=wm[:, :w], lhsT=wr_sb[:],
                                     rhs=prevT[:, s * SEG:s * SEG + w],
                                     start=False, stop=True)
                    nc.scalar.activation(
                        out=outT[:, s * SEG:s * SEG + w], in_=wm[:, :w],
                        func=mybir.ActivationFunctionType.Relu,
                        bias=bias_sb[:, 0:1])

            # ---- layer 1 ----
            layer(xtab, xT, w1l_sb, w1r_sb, b1_sb, h1T, "a")

            # node-major h1 -> ag_in, AllGather into full table
            for nb in range(NB):
                rows = min(128, NPC - 128 * nb)
                tp = tpp.tile([128, 128], f32, name=f"t1{nb}", tag="tp")
                nc.tensor.transpose(out=tp[:rows, :],
                                    in_=h1T[:, 128 * nb:128 * nb + rows],
                                    identity=ident[:])
                nm = ep.tile([128, D], f32, name=f"nm{nb}", tag="nm")
                nc.scalar.activation(out=nm[:rows, :], in_=tp[:rows, :],
                                     func=mybir.ActivationFunctionType.Copy)
                nc.sync.dma_start(out=ag_in[128 * nb:128 * nb + rows, :],
                                  in_=nm[:rows, :])
            nc.gpsimd.collective_compute(
                "AllGather", mybir.AluOpType.bypass,
                replica_groups=[list(range(NCORES))],
                ins=[ag_in[:]], outs=[ag_sh[:]])

            # ---- layer 2: gather directly from the shared AllGather buf ----
            layer(ag_sh, h1T, w2l_sb, w2r_sb, b2_sb, h2T, "b")
            nc.sync.dma_start(out=h_out[:], in_=h2T[:])

            # ---- head: y = h2 @ Wlin + blin ----
            for nb in range(NB):
                rows = min(128, NPC - 128 * nb)
                hp = tpp.tile([128, 128], f32, name=f"hd{nb}", tag="tp")
                nc.tensor.matmul(out=hp[:rows, 0:1],
                                 lhsT=h2T[:, 128 * nb:128 * nb + rows],
                                 rhs=wlin_sb[:, 0:1], start=True, stop=True)
                nc.vector.tensor_scalar_add(
                    out=yrow[:rows, nb:nb + 1], in0=hp[:rows, 0:1],
                    scalar1=blin_sb[:rows, 0:1])
            nc.sync.dma_start(out=y_out[:], in_=yrow[:])
    nc.compile()
    return nc


def kernel(**inputs):
    global LAST
    from concourse.bass_utils import run_bass_kernel_spmd

    x = np.ascontiguousarray(np.asarray(inputs["x"], dtype=np.float32))
    ei = np.asarray(inputs["edge_index"])
    deg = np.bincount(ei[1].astype(np.int64), minlength=N)
    inv_deg = (1.0 / np.maximum(deg, 1)).astype(np.float32)
    nch, metas = _prep(ei, inv_deg)

    key = tuple(nch)
    if key not in _compiled:
        _compiled[key] = _build(nch)
    nc = _compiled[key]

    jt = np.broadcast_to(np.arange(SEG, dtype=np.float32),
                         (128, SEG)).copy()
    w1l = np.asarray(inputs["W1l"], np.float32)
    w1r = np.asarray(inputs["W1r"], np.float32)
    b1 = np.asarray(inputs["b1"], np.float32).reshape(D, 1)
    w2l = np.asarray(inputs["W2l"], np.float32)
    w2r = np.asarray(inputs["W2r"], np.float32)
    b2 = np.asarray(inputs["b2"], np.float32).reshape(D, 1)
    wlin = np.asarray(inputs["Wlin"], np.float32).reshape(D, 1)
    blin = np.full((128, 1), float(np.asarray(inputs["blin"]).reshape(-1)[0]),
                   np.float32)

    in_maps = []
    for c in range(NCORES):
        mi, mo, mv = metas[c]
        in_maps.append({
            "xtab": x, "xown": x[c * NPC:(c + 1) * NPC],
            "msrc": mi, "moff": mo, "minv": mv, "jt": jt,
            "w1l": w1l, "w1r": w1r, "b1": b1,
            "w2l": w2l, "w2r": w2r, "b2": b2,
            "wlin": wlin, "blin": blin,
        })
    LAST = (nc, in_maps)

    res = run_bass_kernel_spmd(nc, in_maps, list(range(NCORES)))
    h_parts, y_parts = [], []
    for c in range(NCORES):
        r = res.results[c]
        h_parts.append(r["h_out"][:, :NPC].T)
        y_parts.append(r["y_out"].T.reshape(-1)[:NPC])
    h = np.ascontiguousarray(np.concatenate(h_parts, axis=0))
    y = np.ascontiguousarray(np.concatenate(y_parts, axis=0))
    return y, h
